# revision 20
# baseline (speedup 1.0000x reference)
"""Trainium2 Bass kernel: BertCL mean-pool + NT-Xent contrastive loss.

Contract: kernel(last_hidden_states [256,512,768] f32, input_mask [256,512] f32)
-> scalar f32 loss, numerically matching the jax reference.

Strategy (8 NeuronCores, SPMD):
  Batch axis sharded STRIDED: core c owns logical batches {c, c+8, c+16, ...}
  (local j <-> logical c + 8j). Only S[0:64, :] is ever needed (rows 0..63 =
  locals j<8 on every core), so:

  stage 1 (memory-bound): per local batch, stream [512,768] through SBUF as a
    [128, 4*768] tile (partition p holds seq rows 4p..4p+3, one contiguous
    12KB chunk per partition -> 128 fat descriptors/batch) and reduce the
    sequence axis with float32r ones-vector matmuls (1 cycle/row vs 4 for
    plain fp32) accumulating in PSUM -> [1,768] sums staged into an SBUF row.
  After local batch 7: AllGather the 8 raw sums [8,768] -> g0 [64,768]
    (= logical rows 0..63 on every core; raw sums suffice because the
    reference's mask division cancels in the L2 normalization). The gather
    and its consume (normalize rows 0..63 with 1/tau folded in, 6 PE
    transposes -> znT) hide under the remaining 24 batches of streaming.
  After local batch 31: normalize the core's own 32 rows, transpose -> zcT,
    and compute ONLY this core's logits columns S_c = zn0 @ znc.T [64,32].
    Diag-mask + exp (Act accum) gives partial denominators [64,1]; the
    strict-upper-triangle partial pair-sums give [64,1]. One tiny [64,2]
    AllReduce(add) replaces the second big AllGather; the replicated finish
    (Ln, cnt-weighted sum, scale) is ~2us. Nothing on the tail moves more
    than 512B through the collective, so in steady state the tail's engine
    work overlaps the next iteration's DMA streaming almost entirely.

  Measured (paired K-differential, sustained calls=5, see perf_lab.py):
  ~177us/iter steady-state vs a ~142-147us measured pure-DMA floor
  (48MiB/core at ~340GB/s sustained) and ~207us for the fp32-matmul
  baseline; relative error on hardware: 2.7e-7. The residual gap over the
  floor is the two CollectiveCompute launches (which hold the DMA-engine
  pool for a mostly fixed duration regardless of payload) plus the s2
  consume chain; SBUF collectives and DVE-issued DMAs are unavailable in
  this bass snapshot, which closes off the cheaper alternatives.

  NOTE: fused DVE ops (tensor_tensor_reduce, scalar_tensor_tensor) pass
  CoreSim but hang/crash this hardware - only plain DVE ops are used.
"""

import sys
from contextlib import ExitStack

import numpy as np

_REPO = "/opt/trn_rl_repo"
if _REPO not in sys.path:
    sys.path.insert(0, _REPO)

import concourse.bass as bass  # noqa: E402  (kept for callers/debugging)
import concourse.tile as tile  # noqa: E402
from concourse import bacc, bass_utils, mybir  # noqa: E402

N_CORES = 8
B, S, H = 256, 512, 768
B_SH = B // N_CORES  # 32 local batches per core
N_PAIR = B // 4  # 64 (= rows of S that matter)
OWN = 8  # locals j<OWN are logical rows < 64
TAU = 0.5
F32 = mybir.dt.float32
AX = mybir.AxisListType
AF = mybir.ActivationFunctionType
NEG = -30000.0  # diagonal mask value; exp(NEG + logit) == 0 exactly in fp32
USE_F32R = True  # stage-1 matmuls in f32r (1 cyc/row) vs fp32 (4 cyc/row)
CONSUME_AT = 19  # emit g0 consume after this batch (gather done by ~b=12)
DUAL_QUEUE = False  # alternate x loads across SP/DVE DGE queues


def _body(
    tc,
    x,
    ident,
    dmask,
    triu,
    cnt,
    out,
    use_collective=True,
    stages=("s1", "mm", "cc", "s2"),
):
    nc = tc.nc

    with ExitStack() as ctx:
        const = ctx.enter_context(tc.tile_pool(name="const", bufs=1))
        ones_col = const.tile([128, 1], F32)
        nc.vector.memset(ones_col[:], 1.0)
        idt = const.tile([128, 128], F32)
        # small const loads go out on the Act queue so the SP queue's first
        # x-stream DMA issues immediately
        nc.scalar.dma_start(idt[:], ident[:])
        dm_sb = const.tile([N_PAIR, B_SH], F32)
        nc.scalar.dma_start(dm_sb[:], dmask[:])
        tri_sb = const.tile([N_PAIR, B_SH], F32)
        nc.scalar.dma_start(tri_sb[:], triu[:])
        cnt_sb = const.tile([1, N_PAIR], F32)
        nc.scalar.dma_start(cnt_sb[:], cnt[:])

        dram = ctx.enter_context(tc.tile_pool(name="dram", bufs=1, space="DRAM"))
        shared = "Shared" if use_collective else "Local"
        cc_in = dram.tile([OWN, H], F32)
        g0 = dram.tile([N_CORES * OWN, H], F32, addr_space=shared, name="g0")
        # AllReduce payload as ONE partition row [1,128]: D_i partials in
        # [0:64], triu partials in [64:128]. Keeps the whole finish on
        # Act+Pool, whose next-iteration work has tens of us of slack, so the
        # collective's latency never stalls PE/DVE/SP between iterations.
        ar_in = dram.tile([1, 2 * N_PAIR], F32)
        ar_out = dram.tile([1, 2 * N_PAIR], F32, addr_space=shared, name="ar_out")

        # staging row for pooled sums: [1, 32*768] on partition 0
        pooled_sb = const.tile([1, B_SH * H], F32)

        xin = ctx.enter_context(tc.tile_pool(name="xin", bufs=6))
        ps1 = ctx.enter_context(tc.tile_pool(name="ps1", bufs=2, space="PSUM"))
        s2 = ctx.enter_context(tc.tile_pool(name="s2", bufs=1))
        s2t = ctx.enter_context(tc.tile_pool(name="s2t", bufs=2))
        psT = ctx.enter_context(tc.tile_pool(name="psT", bufs=2, space="PSUM"))
        psS = ctx.enter_context(tc.tile_pool(name="psS", bufs=1, space="PSUM"))

        znT = s2.tile([128, 6 * N_PAIR], F32)  # zn0.T chunks (h on partitions)
        zcT = s2.tile([128, 6 * B_SH], F32)  # znc.T chunks
        pS = psS.tile([N_PAIR, B_SH], F32)  # S_c = zn0 @ znc.T

        def normalize(zh, P, name):
            """L2-normalize rows in place with 1/sqrt(tau) folded in."""
            sqs = s2t.tile([P, H], F32, tag=f"sqs{name}", name=f"sqs{name}")
            ssn = s2t.tile([P, 1], F32, tag=f"ssn{name}", name=f"ssn{name}")
            nc.vector.tensor_mul(sqs[:], zh[:], zh[:])
            nc.vector.reduce_sum(out=ssn[:], in_=sqs[:], axis=AX.X)
            nrm = s2t.tile([P, 1], F32, tag=f"nrm{name}", name=f"nrm{name}")
            nc.scalar.activation(nrm[:], ssn[:], AF.Sqrt, scale=TAU)
            rn = s2t.tile([P, 1], F32, tag=f"rn{name}", name=f"rn{name}")
            nc.vector.reciprocal(rn[:], nrm[:])
            nc.vector.tensor_scalar_mul(zh[:], zh[:], rn[:, 0:1])

        def transpose_to(dst, zh, P):
            """dst[:, k*P:(k+1)*P] = (zh h-chunk k).T for k in 0..5."""
            for k in range(6):
                pt = psT.tile([128, 128], F32, tag="pt")
                nc.tensor.transpose(
                    pt[:, 0:P], zh[:, k * 128 : (k + 1) * 128], idt[0:P, 0:P]
                )
                nc.vector.tensor_copy(dst[:, k * P : (k + 1) * P], pt[:, 0:P])

        def send_g0():
            """Gather raw sums of locals 0..7 (= logical rows 0..63)."""
            nc.scalar.dma_start(
                cc_in[:],
                pooled_sb[0:1, 0 : OWN * H].rearrange("o (b e) -> o b e", e=H),
            )
            if use_collective:
                nc.gpsimd.collective_compute(
                    "AllGather",
                    mybir.AluOpType.bypass,
                    replica_groups=[list(range(N_CORES))],
                    ins=[cc_in[:].opt()],
                    outs=[g0[:].opt()],
                )
            else:
                for c in range(N_CORES):
                    nc.sync.dma_start(g0[c * OWN : (c + 1) * OWN, :], cc_in[:])

        def consume_g0():
            """Normalize logical rows 0..63 and fill znT.

            Gathered row (c, j) holds logical batch c + 8j; the permuted 3-D
            AP (j, c, e) lands partition p = 8j + c = logical index."""
            zh = s2.tile([N_PAIR, H], F32, tag="zb0", name="zb0")
            nc.scalar.dma_start(zh[:], g0.rearrange("(c j) e -> j c e", c=N_CORES))
            normalize(zh, N_PAIR, "g")
            transpose_to(znT, zh, N_PAIR)

        def own_chain():
            """S_c, partial denoms + pair-sums, tiny AllReduce."""
            zc = s2.tile([B_SH, H], F32, tag="zc", name="zc")
            nc.scalar.dma_start(
                zc[:], pooled_sb[0:1, :].rearrange("o (b e) -> o b e", e=H)
            )
            normalize(zc, B_SH, "c")
            transpose_to(zcT, zc, B_SH)
            for k in range(6):
                nc.tensor.matmul(
                    pS[:],
                    lhsT=znT[:, k * N_PAIR : (k + 1) * N_PAIR],
                    rhs=zcT[:, k * B_SH : (k + 1) * B_SH],
                    start=(k == 0),
                    stop=(k == 5),
                )
            sd = s2.tile([N_PAIR, B_SH], F32)
            nc.vector.tensor_add(sd[:], pS[:], dm_sb[:])
            # logits are cosine/tau in [-2,2]: exp safe without max-subtract
            et = s2.tile([N_PAIR, B_SH], F32)
            ar_sb = s2.tile([N_PAIR, 2], F32)
            nc.scalar.activation(
                et[:], sd[:], AF.Exp, scale=1.0, accum_out=ar_sb[:, 0:1]
            )
            mt = s2.tile([N_PAIR, B_SH], F32)
            nc.vector.tensor_mul(mt[:], sd[:], tri_sb[:])
            nc.vector.reduce_sum(out=ar_sb[:, 1:2], in_=mt[:], axis=AX.X)
            # partition-gather both columns into the [1,128] payload row
            nc.scalar.dma_start(
                ar_in[0:1, 0:N_PAIR].rearrange("o (p e) -> o p e", e=1),
                ar_sb[:, 0:1],
            )
            nc.scalar.dma_start(
                ar_in[0:1, N_PAIR : 2 * N_PAIR].rearrange("o (p e) -> o p e", e=1),
                ar_sb[:, 1:2],
            )
            if use_collective:
                nc.gpsimd.collective_compute(
                    "AllReduce",
                    mybir.AluOpType.add,
                    replica_groups=[list(range(N_CORES))],
                    ins=[ar_in[:].opt()],
                    outs=[ar_out[:].opt()],
                )
            else:
                nc.sync.dma_start(ar_out[:], ar_in[:])

        def finish():
            # single-partition finish on Act + Pool only (no PE/DVE/SP):
            # the AllReduce latency lands on streams with cross-iteration slack
            fin = s2.tile([1, 2 * N_PAIR], F32)
            nc.scalar.dma_start(fin[:], ar_out[:])
            ld = s2.tile([1, N_PAIR], F32)
            nc.scalar.activation(ld[:], fin[0:1, 0:N_PAIR], AF.Ln)  # logden
            t1 = s2.tile([1, N_PAIR], F32)
            nc.gpsimd.tensor_mul(t1[:], ld[:], cnt_sb[:])
            pr = s2.tile([1, N_PAIR], F32)
            nc.gpsimd.tensor_sub(pr[:], t1[:], fin[0:1, N_PAIR : 2 * N_PAIR])
            tot = s2.tile([1, 1], F32)
            nc.gpsimd.reduce_sum(out=tot[:], in_=pr[:], axis=AX.XYZWC)
            res = s2.tile([1, 1], F32)
            nc.gpsimd.tensor_scalar_mul(
                res[:], tot[:], -2.0 / N_PAIR * (N_PAIR - 1)
            )
            nc.scalar.dma_start(out[0:1, 0:1], res[:])

        # ---- stage 1: per-batch sum over the sequence axis -------------------
        # float32r matmuls: same 4-byte operands, but the PE processes the
        # moving tensor at 1 cycle/row (vs 4 for plain fp32) when the output
        # free size is >= 256 — stage-1 PE time drops ~4x, below the DMA
        # stream time, so the kernel is memory-bound as intended.
        F32R = mybir.dt.float32r
        # partition p <- seq rows 4p..4p+3: per-partition data is ONE
        # contiguous 12KB chunk (vs 4 strided 3KB chunks for "(c p)"), so the
        # DMA needs 128 descriptors/batch instead of 512. The matmul code is
        # unchanged: summing rows {4p+c} over p then c still sums all 512.
        x4 = x.rearrange("b (p c) e -> b p c e", c=4)  # [32, 128, 4, 768]
        for b in range(B_SH):
            if "s1" in stages:
                xt = xin.tile([128, 4 * H], F32)
                # out AP typed f32r so the BIR verifier accepts the f32r
                # matmul consumers (PE rounds on ingest; same 4-byte words)
                rdt = F32R if USE_F32R else F32
                eng = nc.vector if (DUAL_QUEUE and b % 2) else nc.sync
                eng.dma_start(xt[:].bitcast(rdt), x4[b].bitcast(rdt))
                if "mm" in stages:
                    ps = ps1.tile([1, H], F32)
                    for c in range(4):
                        nc.tensor.matmul(
                            ps[:, 0:512],
                            lhsT=ones_col[:, 0:1].bitcast(rdt),
                            rhs=xt[:, c * H : c * H + 512].bitcast(rdt),
                            start=(c == 0),
                            stop=(c == 3),
                        )
                    for c in range(4):
                        nc.tensor.matmul(
                            ps[:, 512:H],
                            lhsT=ones_col[:, 0:1].bitcast(rdt),
                            rhs=xt[:, c * H + 512 : (c + 1) * H].bitcast(rdt),
                            start=(c == 0),
                            stop=(c == 3),
                        )
                    nc.scalar.copy(pooled_sb[0:1, b * H : (b + 1) * H], ps[:])
            if "cc" in stages:
                if b == OWN - 1:
                    send_g0()
                elif b == CONSUME_AT and "s2" in stages:
                    consume_g0()

        if "cc" not in stages or "s2" not in stages:
            return
        own_chain()
        finish()


def build_nc():
    nc = bacc.Bacc("TRN2", target_bir_lowering=False, debug=False, num_devices=N_CORES)
    x = nc.dram_tensor("x", [B_SH, S, H], F32, kind="ExternalInput")
    ident = nc.dram_tensor("ident", [128, 128], F32, kind="ExternalInput")
    dmask = nc.dram_tensor("dmask", [N_PAIR, B_SH], F32, kind="ExternalInput")
    triu = nc.dram_tensor("triu", [N_PAIR, B_SH], F32, kind="ExternalInput")
    cnt = nc.dram_tensor("cnt", [1, N_PAIR], F32, kind="ExternalInput")
    out = nc.dram_tensor("loss", [1, 1], F32, kind="ExternalOutput")
    with tile.TileContext(nc) as tc:
        _body(
            tc,
            x.ap(),
            ident.ap(),
            dmask.ap(),
            triu.ap(),
            cnt.ap(),
            out.ap(),
        )
    nc.compile()
    return nc


def const_inputs(core):
    """Per-core constants: local column jl holds logical batch core + 8*jl."""
    ident = np.eye(128, dtype=np.float32)
    dmask = np.zeros((N_PAIR, B_SH), dtype=np.float32)
    triu = np.zeros((N_PAIR, B_SH), dtype=np.float32)
    for jl in range(OWN):  # only logical < 64 can hit the diag/pair window
        lg = core + N_CORES * jl
        dmask[lg, jl] = NEG
        triu[:lg, jl] = 1.0  # pair (i, lg) for i < lg
    cnt = (N_PAIR - 1 - np.arange(N_PAIR, dtype=np.float32)).reshape(1, N_PAIR)
    return {"ident": ident, "dmask": dmask, "triu": triu, "cnt": cnt}


def make_in_maps(last_hidden_states, input_mask):
    del input_mask  # cancels exactly in the L2 normalization (see _body)
    x = np.asarray(last_hidden_states, dtype=np.float32)
    return [
        {"x": np.ascontiguousarray(x[c::N_CORES]), **const_inputs(c)}
        for c in range(N_CORES)
    ]


_CACHE = {}


def get_nc():
    if "nc" not in _CACHE:
        _CACHE["nc"] = build_nc()
    return _CACHE["nc"]


def kernel(last_hidden_states, input_mask):
    nc = get_nc()
    in_maps = make_in_maps(last_hidden_states, input_mask)
    res = bass_utils.run_bass_kernel_spmd(nc, in_maps, core_ids=list(range(N_CORES)))
    return np.asarray(res.results[0]["loss"], dtype=np.float32).reshape(())
